# revision 12
# baseline (speedup 1.0000x reference)
"""MoE FFN (top-2 of 8 experts) Trainium2 kernel — F-slice parallel.

Sharding: every core processes the SAME dispatched token stream (all 8
experts' gathered tokens, 16384 token-assignments total), but only a
512-wide slice of the FFN hidden dimension F (Megatron-style column-
parallel W1 / row-parallel W2).  Core c holds F columns/rows
[c*512, (c+1)*512) of every expert.  Each core emits a partial output
(its F-slice's contribution, bf16); the host sums the 8 partials and
applies combine weights + b2.

Why: per-expert token counts are imbalanced (1932..2182 for the seed-0
router), so expert-parallel cores pad to the max count.  F-slicing makes
per-core work exactly sum(c_e)/8 = 2048 token-equivalents — perfectly
balanced, zero padding.

Device loop per (expert, ~448-token chunk), two phases:
    phase 1:  hT[f,:] = gelu(sum_k W1[k,f].T @ xT[k,:])   4 f-tiles
    phase 2:  yT[hs,:] += W2[f,hs].T @ hT[f,:] over the 4 f-tiles,
              accumulated in PSUM (fp32), 2 passes (6+2 h-subtiles)
              -> one bf16 partial-y store per chunk.

Matmuls in bf16 (full rate, FWL); PSUM fp32.  rel-err ~4e-3 << 2e-2.
"""

import os
import sys
import numpy as np

for _p in ("/opt/trn_rl_repo", "/root/.axon_site/_ro/trn_rl_repo"):
    if _p not in sys.path and os.path.isdir(_p):
        sys.path.append(_p)

import concourse.bacc as bacc  # noqa: E402
import concourse.tile as tile  # noqa: E402
from concourse import mybir  # noqa: E402
from concourse.bass_utils import run_bass_kernel_spmd  # noqa: E402

# Problem shapes (hardcoded per spec)
B, S, H, F, E = 4, 2048, 1024, 4096, 8
T = B * S
TOP_K = 2
N_CORES = 8
P = 128
KH = H // P          # 8   H-contraction subtiles
FT = 32              # f-tiles resident per core (8 experts x 4)
FU = 4               # f-tiles per expert per core (F-slice = 512)
FSL = FU * P         # 512 F columns per expert per core
HS = H // P          # 8   output H subtiles
CH = 512             # max token chunk
WG = 4               # f-tiles per weight-stream DMA slice
TT = 2 * T           # total token-assignments (top-2)

F32 = mybir.dt.float32
BF16 = mybir.dt.bfloat16

_CACHE: dict = {}
LAST_RESULT = None  # BassKernelResults of the most recent run (for test.py)


def _split(c: int) -> list:
    """Balanced chunk sizes <= CH (avoids tiny-N matmul tails)."""
    n = -(-c // CH)
    base, r = divmod(c, n)
    return [base + 1] * r + [base] * (n - r)


def _build(counts: tuple, use_b1: bool, mm_dt):
    nc = bacc.Bacc(
        "TRN2",
        target_bir_lowering=False,
        debug=False,
        enable_asserts=False,
        num_devices=N_CORES,
    )

    # flat chunk plan: (xd index, f-tile base, token count, y offset)
    plan = []
    yoff = 0
    for e in range(E):
        for nt in _split(counts[e]):
            plan.append((len(plan), FU * e, nt, yoff))
            yoff += nt
    nch = len(plan)
    assert yoff == sum(counts)

    xd = nc.dram_tensor("xd", [P, nch, KH, CH], mm_dt, kind="ExternalInput").ap()
    # w1d[p, 4e+j, k, q] = W1[e][k*128+p, c*512 + j*128 + q]   (core c)
    w1d = nc.dram_tensor("w1d", [P, FT, KH, P], mm_dt, kind="ExternalInput").ap()
    # w2d[p, 4e+j, h]   = W2[e][c*512 + j*128 + p, h]
    w2d = nc.dram_tensor("w2d", [P, FT, H], mm_dt, kind="ExternalInput").ap()
    if use_b1:
        b1d = nc.dram_tensor("b1d", [P, FT], F32, kind="ExternalInput").ap()
    # yd[p, ci, hs, t] = partial_y chunk ci  (bf16; host sums the 8 cores)
    yd = nc.dram_tensor("yd", [P, nch, HS, CH], BF16, kind="ExternalOutput").ap()

    gelu = mybir.ActivationFunctionType.Gelu_apprx_tanh

    with tile.TileContext(nc) as tc:
        with (
            tc.tile_pool(name="w1p", bufs=1) as w1p,
            tc.tile_pool(name="w2p", bufs=1) as w2p,
            tc.tile_pool(name="xp", bufs=6) as xp,
            tc.tile_pool(name="hp", bufs=FU + 1) as hp,
            tc.tile_pool(name="yp", bufs=2) as yp,
            tc.tile_pool(name="bp", bufs=1) as bp,
            tc.tile_pool(name="pp", bufs=1, space="PSUM") as pp,
        ):
            # PE clock warm-up during the initial DMA wait (HAM gate sits at
            # 1.2 GHz until ~3.4us of sustained activity).
            wsrc = bp.tile([P, CH], mm_dt, name="warm_src")
            nc.vector.memset(wsrc[:, :P], 0.0)
            wdst = pp.tile([P, CH], F32, tag="pt", bufs=2, name="warm_dst")
            for _ in range(48):
                nc.tensor.matmul(
                    wdst[:, :P], wsrc[:, :P], wsrc[:, :P], start=True, stop=True
                )

            # All DMA issue on the sync (SP) ring — keeps the scalar queue
            # free for the gelu ACT-table load + activations.
            if use_b1:
                b1t = bp.tile([P, FT], F32)
                nc.sync.dma_start(b1t[:], b1d[:])

            xts: dict = {}

            def _x_load(ci):
                xts[ci] = xp.tile([P, KH, CH], mm_dt, tag="xt",
                                  name=f"xt_{ci}")
                nc.sync.dma_start(xts[ci][:], xd[:, ci])

            xts[0] = xp.tile([P, KH, CH], mm_dt, tag="xt", name="xt_0")
            nc.sync.dma_start(xts[0][:, :4], xd[:, 0, :4])

            # resident weights, streamed in slices; first two w1 slices fine
            # (f0 alone, then f1-3) so GEMM1 starts after ~0.75 MiB
            w1sl = [(0, 1), (1, 4)] + [(g, g + WG) for g in range(WG, FT, WG)]
            w2sl = [(g, g + WG) for g in range(0, FT, WG)]
            w1map: list = [None] * FT
            w2map: list = [None] * FT

            def _w1_load(si):
                lo, hi = w1sl[si]
                t = w1p.tile([P, hi - lo, KH, P], mm_dt, tag=f"w1g{si}",
                             name=f"w1g_{si}", bufs=1)
                nc.sync.dma_start(t[:], w1d[:, lo:hi])
                for f in range(lo, hi):
                    w1map[f] = (t, f - lo)

            def _w2_load(si):
                lo, hi = w2sl[si]
                t = w2p.tile([P, hi - lo, H], mm_dt, tag=f"w2g{si}",
                             name=f"w2g_{si}", bufs=1)
                nc.sync.dma_start(t[:], w2d[:, lo:hi])
                for f in range(lo, hi):
                    w2map[f] = (t, f - lo)

            _w1_load(0)
            _w1_load(1)
            nc.sync.dma_start(xts[0][:, 4:], xd[:, 0, 4:])
            _w2_load(0)
            for pi in range(1, 5):
                if pi < nch:
                    _x_load(pi)
                _w1_load(pi + 1)
                _w2_load(pi)
            for si in range(6, len(w1sl)):
                _w1_load(si)
                _w2_load(si - 1)

            for ci, fbase, nt, yo in plan:
                xt = xts.pop(ci)
                if ci + 5 < nch:
                    _x_load(ci + 5)

                # phase 1: hT = gelu(W1.T @ xT) for this expert's 4 f-tiles
                hqs = []
                for fi in range(FU):
                    f = fbase + fi
                    w1t, j = w1map[f]
                    pt = pp.tile([P, CH], F32, tag="pt", bufs=2)
                    for k in range(KH):
                        nc.tensor.matmul(
                            pt[:, :nt],
                            w1t[:, j, k, :],
                            xt[:, k, :nt],
                            start=(k == 0),
                            stop=(k == KH - 1),
                        )
                    hq = hp.tile([P, CH], mm_dt, tag="hq", name=f"hq_{ci}_{fi}")
                    bias = b1t[:, f : f + 1] if use_b1 else 0.0
                    nc.scalar.activation(hq[:, :nt], pt[:, :nt], gelu, bias=bias)
                    hqs.append(hq)

                # phase 2: partial yT[hs] = sum_f W2[f,hs].T @ hT[f] in PSUM;
                # two passes (6+2 h-subtiles) to fit 8 PSUM banks.
                yt = yp.tile([P, HS, CH], BF16, tag="yt")

                def gemm2_pass(hs_list):
                    pys = [
                        pp.tile([P, CH], F32, tag="py", bufs=6,
                                name=f"py_{ci}_{hs}")
                        for hs in hs_list
                    ]
                    for fi in range(FU):
                        f = fbase + fi
                        w2t, j = w2map[f]
                        for pi, hs in enumerate(hs_list):
                            nc.tensor.matmul(
                                pys[pi][:, :nt],
                                w2t[:, j, hs * P : (hs + 1) * P],
                                hqs[fi][:, :nt],
                                start=(fi == 0),
                                stop=(fi == FU - 1),
                            )
                    for pi, hs in enumerate(hs_list):
                        if pi % 2 == 0:
                            nc.vector.tensor_copy(yt[:, hs, :nt], pys[pi][:, :nt])
                        else:
                            nc.scalar.copy(yt[:, hs, :nt], pys[pi][:, :nt])

                gemm2_pass(list(range(6)))
                gemm2_pass([6, 7])
                # y goes out on the scalar HWDGE ring: the sync ring's FIFO
                # is occupied by the weight+x stream for the first ~70us, and
                # a queued-up y DMA would block yt pool recycling (=> PE).
                nc.scalar.dma_start(yd[:, ci], yt[:])

    nc.compile()
    return nc


def _route(x2d, Wg):
    """Replicates reference router: softmax -> top-2 -> renormalize."""
    logits = x2d @ Wg  # [T, E] fp32
    m = logits.max(axis=-1, keepdims=True)
    p = np.exp(logits - m, dtype=np.float32)
    p /= p.sum(axis=-1, keepdims=True)
    # jax.lax.top_k: values descending, ties broken by lower index.
    order = np.argsort(-p, axis=-1, kind="stable")
    top_i = order[:, :TOP_K]  # [T, 2]
    top_p = np.take_along_axis(p, top_i, axis=-1)
    top_p = top_p / top_p.sum(axis=-1, keepdims=True)
    return top_i, top_p


def kernel(x, Wg, W1, b1, W2, b2):
    global LAST_RESULT
    x = np.ascontiguousarray(np.asarray(x, dtype=np.float32))
    Wg = np.ascontiguousarray(np.asarray(Wg, dtype=np.float32))
    W1 = np.ascontiguousarray(np.asarray(W1, dtype=np.float32))
    b1 = np.ascontiguousarray(np.asarray(b1, dtype=np.float32))
    W2 = np.ascontiguousarray(np.asarray(W2, dtype=np.float32))
    b2 = np.ascontiguousarray(np.asarray(b2, dtype=np.float32))

    x2d = x.reshape(T, H)
    top_i, top_p = _route(x2d, Wg)

    rows = [None] * E
    gval = [None] * E
    for e in range(E):
        r, slot = np.nonzero(top_i == e)
        rows[e] = r
        gval[e] = top_p[r, slot]

    counts = tuple(len(r) for r in rows)
    use_b1 = bool(np.any(b1))

    mm_dt = {
        "bf16": BF16,
        "fp32": F32,
    }[os.environ.get("KERNEL_MMDT", "bf16")]
    key = (counts, use_b1, str(mm_dt))
    if key not in _CACHE:
        _CACHE[key] = _build(counts, use_b1, mm_dt)
    nc = _CACHE[key]

    np_dt = mybir.dt.np(mm_dt)

    # xd: identical for every core — all experts' gathered tokens, packed
    # chunk-contiguous.
    nch = sum(-(-c // CH) for c in counts)
    xd = np.zeros((P, nch, KH, CH), np_dt)
    ci = 0
    for e in range(E):
        xe = x2d[rows[e]].T.astype(np_dt)  # [H, c_e]
        xk = xe.reshape(KH, P, counts[e])
        coff = 0
        for nt in _split(counts[e]):
            xd[:, ci, :, :nt] = xk[:, :, coff : coff + nt].transpose(1, 0, 2)
            coff += nt
            ci += 1
    xd = np.ascontiguousarray(xd)

    in_maps = []
    for c in range(N_CORES):
        lo, hi = c * FSL, (c + 1) * FSL
        # W1 slices: [E] x [H, 512] -> [P, FT, KH, P]
        w1c = np.empty((P, FT, KH, P), np_dt)
        w2c = np.empty((P, FT, H), np_dt)
        for e in range(E):
            w1s = W1[e][:, lo:hi].astype(np_dt)  # [H, 512]
            w1c[:, FU * e : FU * (e + 1)] = (
                w1s.reshape(KH, P, FU, P).transpose(1, 2, 0, 3)
            )
            w2s = W2[e][lo:hi].astype(np_dt)  # [512, H]
            w2c[:, FU * e : FU * (e + 1)] = (
                w2s.reshape(FU, P, H).transpose(1, 0, 2)
            )
        m = {
            "xd": xd,
            "w1d": np.ascontiguousarray(w1c),
            "w2d": np.ascontiguousarray(w2c),
        }
        if use_b1:
            b1c = np.empty((P, FT), np.float32)
            for e in range(E):
                b1c[:, FU * e : FU * (e + 1)] = b1[e][lo:hi].reshape(FU, P).T
            m["b1d"] = np.ascontiguousarray(b1c)
        in_maps.append(m)

    trace = os.environ.get("KERNEL_TRACE", "") == "1"
    res = run_bass_kernel_spmd(
        nc,
        in_maps,
        core_ids=list(range(N_CORES)),
        trace=trace,
        trace_cores=[0] if trace else None,
    )
    LAST_RESULT = res

    # sum the 8 partial outputs, then combine
    ysum = res.results[0]["yd"].astype(np.float32)  # [P, nch, HS, CH]
    for c in range(1, N_CORES):
        ysum += res.results[c]["yd"].astype(np.float32)
    # [P, nch, HS, CH] -> per chunk [nt, H]
    out = np.zeros((T, H), np.float32)
    ci = 0
    for e in range(E):
        ye = np.empty((counts[e], H), np.float32)
        coff = 0
        for nt in _split(counts[e]):
            ye[coff : coff + nt] = (
                ysum[:, ci, :, :nt].transpose(2, 1, 0).reshape(nt, H)
            )
            coff += nt
            ci += 1
        out[rows[e]] += gval[e][:, None] * (ye + b2[e][None, :])

    return out.reshape(B, S, H)
